# revision 8
# baseline (speedup 1.0000x reference)
"""Trainium2 kernel for nn_AdaFastFoodMergedModel.

FastFood transform: y = SCALE * Sel . H . diag(G) . Pi . H . diag(B) . x
(H = 4096-point orthonormal Walsh-Hadamard, Pi = random permutation,
Sel = row subset of size 1228).

Strategy: everything right of `x` is a fixed linear operator built from the
small inputs (B, G, Pi, row_idx), so fold it on the host into one dense
matrix W [4096, 1228] (bf16) and run y = x @ W on the TensorEngine.
Device work per core (rows sharded 8192/8 = 1024 rows):
  - DMA W as 8 separate group tiles [128, 4, 1228] so matmuls only wait on
    the group they need (one monolithic tile serialized the whole start)
  - stage DMA issue to stay within the 8 DMA-semaphore lanes: the previous
    version issued 14 DMAs up-front and lanes 9+ serialized behind lane
    recycling, starving the PE at t=29-70us and letting HAM re-throttle
  - x row tiles on the gpsimd (SWDGE) ring, W + late x tiles on scalar,
    transposes on sync -- three independent queues share the 16 SDMA engines
  - cast to bf16 (DVE, 2 chunks/tile) + xbar transpose (2 per tile)
  - kc-outer matmul loop: 3 PSUM accumulators stay open across kc so the
    lhsT (xT chunk) is loaded once per kc (LDW fully hidden under 3 matmuls)
  - evacuate psum -> SBUF split across DVE/ACT, DMA out f32 on gpsimd
No cross-core communication (data parallel over rows).
"""

import math
import sys

import numpy as np

sys.path.insert(0, "/opt/trn_rl_repo")

import ml_dtypes

ROWS, D = 8192, 4096
M = 1228
SCALE = math.sqrt(D / M)
N_CORES = 8
SHARD = ROWS // N_CORES  # 1024
P = 128
KC = D // P  # 32 contraction chunks
RT = SHARD // P  # 8 row tiles per core
SEL_CHUNKS = [(0, 512), (512, 512), (1024, 204)]  # 1228 = 512+512+204
WGROUPS = 8
WG = KC // WGROUPS  # 4 kc per W group

# set by test harness to collect a profile
TRACE = False
LAST = {}

_CACHE = {}


def _fwht_cols(a: np.ndarray) -> np.ndarray:
    """Orthonormal FWHT along axis 0 (Sylvester/natural order)."""
    n = a.shape[0]
    x = a.copy()
    h = 1
    while h < n:
        x = x.reshape(n // (2 * h), 2, h, -1)
        lo = x[:, 0]
        hi = x[:, 1]
        x = np.stack((lo + hi, lo - hi), axis=1).reshape(n, -1)
        h *= 2
    return x * (1.0 / math.sqrt(n))


def _build_w(B, G, Pi, row_idx) -> np.ndarray:
    """W such that y = x @ W  (float32)."""
    S = np.zeros((D, M), dtype=np.float64)
    S[row_idx, np.arange(M)] = 1.0  # Sel^T
    A = _fwht_cols(S)  # H .
    A = A * G[:, None].astype(np.float64)  # diag(G) .
    A2 = np.empty_like(A)
    A2[Pi] = A  # Pi^T .
    A2 = _fwht_cols(A2)  # H .
    W = SCALE * (B[:, None].astype(np.float64) * A2)  # diag(B) .
    return W.astype(np.float32)


def _install_ntff_shim():
    """The image's antenv lacks axon_hooks; provide it so
    run_bass_kernel_spmd(trace=True) can collect an NTFF profile."""
    import types

    try:
        import antenv.axon_hooks  # noqa: F401

        return
    except ImportError:
        pass
    try:
        from trn_agent_boot.trn_boot import _ntff_profile_via_ctypes

        hook = _ntff_profile_via_ctypes("/opt/axon/libaxon_pjrt.so")
    except Exception:
        hook = None
    mod = types.ModuleType("antenv.axon_hooks")
    mod.get_axon_ntff_profile_hook = lambda: hook
    mod.set_axon_ntff_profile_hook = lambda h: None
    sys.modules["antenv.axon_hooks"] = mod


def _build_bass():
    import concourse.bass as bass
    import concourse.bacc as bacc
    import concourse.mybir as mybir
    from concourse import tile

    f32 = mybir.dt.float32
    bf16 = mybir.dt.bfloat16

    nc = bacc.Bacc("TRN2", target_bir_lowering=False, debug=False)
    x_in = nc.declare_dram_parameter("x", [SHARD, D], f32, isOutput=False)
    # W pre-arranged on host to the SBUF layout [p, kc, m] so each DMA is
    # contiguous per partition
    w_in = nc.declare_dram_parameter("w", [P, KC, M], bf16, isOutput=False)
    out = nc.declare_dram_parameter("out", [SHARD, M], f32, isOutput=True)

    CH = 4  # column-chunks per row tile for cast/transpose
    CW = D // CH  # 2048 features per chunk
    KCC = KC // CH  # 16 k-chunks per column-chunk

    with tile.TileContext(nc) as tc:
        with (
            tc.tile_pool(name="const", bufs=1) as const_pool,
            tc.tile_pool(name="xf", bufs=4) as xf_pool,
            tc.tile_pool(name="xbf", bufs=2) as xbf_pool,
            tc.tile_pool(name="xT", bufs=3) as xT_pool,
            tc.tile_pool(name="y", bufs=2) as y_pool,
            tc.tile_pool(name="psy", bufs=2, space=bass.MemorySpace.PSUM) as psy_pool,
        ):
            w_tiles = [
                const_pool.tile([P, WG, M], bf16, tag=f"w{g}", name=f"w{g}")
                for g in range(WGROUPS)
            ]

            def emit_w(g):
                nc.scalar.dma_start(
                    w_tiles[g][:], w_in[:, g * WG : (g + 1) * WG, :]
                )

            def emit_load(rt, ring="gpsimd"):
                """x row-tile load, column-chunked so casts start early."""
                eng = getattr(nc, ring)
                if rt == 0:
                    chunks = []
                    for c in range(4):
                        xfc = const_pool.tile([P, D // 4], f32, tag=f"xf0c{c}")
                        eng.dma_start(
                            xfc[:], x_in[0:P, c * (D // 4) : (c + 1) * (D // 4)]
                        )
                        chunks.append(xfc)
                    return chunks
                xf = xf_pool.tile([P, D], f32, tag="xf")
                eng.dma_start(xf[:], x_in[rt * P : (rt + 1) * P, :])
                return xf

            def emit_prep(xf):
                """Chunked cast (DVE) + xbar transpose (sync ring).
                xT[c][p, k, j] = x[j, (c*KCC+k)*128+p] in bf16."""
                xTc = []
                for c in range(CH):
                    xtb = xbf_pool.tile([P, CW], bf16, tag=f"xtb{c}")
                    if isinstance(xf, list):
                        nc.vector.tensor_copy(xtb[:], xf[c][:])
                    else:
                        nc.vector.tensor_copy(xtb[:], xf[:, c * CW : (c + 1) * CW])
                    xT = xT_pool.tile([P, KCC, P], bf16, tag=f"xT{c}")
                    nc.sync.dma_start(xT[:], xtb[:], transpose=True)
                    xTc.append(xT)
                return xTc

            # --- staged issue: stay within the 8 DMA sem lanes ---
            xfs = {0: emit_load(0)}          # 4 DMAs (gpsimd)
            emit_w(0)                         # scalar
            emit_w(1)
            xfs[1] = emit_load(1, "scalar")
            xTc = emit_prep(xfs[0])           # casts + 2 transposes (sync)
            emit_w(2)
            xfs[2] = emit_load(2, "gpsimd")
            emit_w(3)

            for rt in range(RT):
                # just-in-time issue of remaining W groups / x prefetches
                if rt == 0:
                    emit_w(4)
                if 0 < rt and rt + 2 < RT:
                    xfs[rt + 2] = emit_load(rt + 2, "gpsimd" if rt % 2 else "scalar")
                if rt + 1 < RT:
                    next_xTc = emit_prep(xfs[rt + 1])

                psys = []
                for i, (off, sz) in enumerate(SEL_CHUNKS):
                    psys.append(
                        psy_pool.tile([P, sz], f32, tag=f"psy{i}", name=f"psy{i}")
                    )
                for kc in range(KC):
                    # stagger remaining W-group issue within rt0 so each is
                    # emitted (program order) well before its first use and
                    # DMA-lane pressure stays under the 8-lane budget
                    if rt == 0 and kc in (4, 8, 12):
                        emit_w(4 + kc // 4)
                    lhsT = xTc[kc // KCC][:, kc % KCC, :]
                    wsl = w_tiles[kc // WG]
                    for i, (off, sz) in enumerate(SEL_CHUNKS):
                        nc.tensor.matmul(
                            psys[i][:],
                            lhsT,
                            wsl[:, kc % WG, off : off + sz],
                            start=(kc == 0),
                            stop=(kc == KC - 1),
                        )
                y_sb = y_pool.tile([P, M], f32)
                nc.vector.tensor_copy(y_sb[:, 0:512], psys[0][:])
                nc.scalar.copy(y_sb[:, 512:1024], psys[1][:])
                nc.vector.tensor_copy(y_sb[:, 1024:1228], psys[2][:])
                nc.gpsimd.dma_start(out[rt * P : (rt + 1) * P, :], y_sb[:])
                if rt + 1 < RT:
                    xTc = next_xTc

    nc.compile()
    return nc


def kernel(x, B, G, Pi, row_idx):
    x = np.ascontiguousarray(np.asarray(x, dtype=np.float32))
    B = np.asarray(B, dtype=np.float32)
    G = np.asarray(G, dtype=np.float32)
    Pi = np.asarray(Pi, dtype=np.int32)
    row_idx = np.asarray(row_idx, dtype=np.int32)

    W = _build_w(B, G, Pi, row_idx).astype(ml_dtypes.bfloat16)
    # rearrange to SBUF layout [p, kc, m]: W[kc*128+p, m] -> Wp[p, kc, m]
    Wp = np.ascontiguousarray(W.reshape(KC, P, M).transpose(1, 0, 2))

    if "nc" not in _CACHE:
        _CACHE["nc"] = _build_bass()
    nc = _CACHE["nc"]

    if TRACE:
        _install_ntff_shim()

    from concourse.bass_utils import run_bass_kernel_spmd

    shards = [x[i * SHARD : (i + 1) * SHARD] for i in range(N_CORES)]
    in_maps = [{"x": shards[i], "w": Wp} for i in range(N_CORES)]

    res = run_bass_kernel_spmd(
        nc, in_maps, core_ids=list(range(N_CORES)), trace=TRACE
    )
    LAST["exec_time_ns"] = getattr(res, "exec_time_ns", None)
    LAST["results"] = res

    outs = [np.asarray(res.results[i]["out"]) for i in range(N_CORES)]
    return np.concatenate(outs, axis=0).astype(np.float32)


if __name__ == "__main__":
    rng = np.random.default_rng(0)
    x = rng.standard_normal((ROWS, D), dtype=np.float32)
    B = (rng.integers(0, 2, D) * 2 - 1).astype(np.float32)
    G = rng.standard_normal(D, dtype=np.float32)
    Pi = rng.permutation(D).astype(np.int32)
    row_idx = rng.permutation(D)[:M].astype(np.int32)
    y = kernel(x=x, B=B, G=G, Pi=Pi, row_idx=row_idx)
    print("out", y.shape, y.dtype)
